# revision 45
# baseline (speedup 1.0000x reference)
"""Trainium2 Bass kernel for ViT-style attention with continuous relative
position bias (nn_Attention_18554258718870).

Sharding: data-parallel over batch B=64 across 8 NeuronCores (8 batches per
core); weights / bias table replicated.

Host side computes the tiny bias-table MLP (961x2 -> 961x12) and the
idx_table gather; the [kv, h, q] bias tensor is shipped to the device and
added into the attention scores on-chip via identity matmuls.

Device-side structure (per batch of 8 per core):
- x loaded with one DMA as [65, 4*768]; PE transposes to xT [768, 260].
- q/k projection in transposed layout (12 tiles of [128, 260] f32r).
- v projection in natural layout, interleaved per head as [v_h | ones_h]
  bf16 so the PV matmul computes the softmax denominator for free.
- scores: per head-pair and kv-chunk, ONE 2-bank PSUM slot holds both
  heads' [kv_chunk, 260] score tiles (cols 0:260 / 512:772); the bias is
  accumulated by a bf16 identity matmul (start=True) before the K=64 f32r
  score matmuls (the two heads sit in disjoint PE row-groups 0/64).
  ONE exp per chunk (Act) covers both heads -> probs bf16 [*, 520].
- PV accumulates [d | denom] x probs into one PSUM bank per head; DVE
  stages the denominator to SBUF (reciprocal_approx_fast cannot read PSUM
  on HW - it crashes the NEFF), takes the fast reciprocal and scales.
- output projection computes the natural [tok, 768] layout directly
  (lhsT = aoT chunk, both 512/256 halves through adjacent matmuls sharing
  the stationary operand); the projection bias is added during PSUM
  evacuation against a 128-row replicated bias tile; DMA out per chunk.

PSUM: 2x 2-bank score slots + 4x 1-bank general slots (PV pairs +
projection pipeline). Engine balance: Act = exps + qk/xT evacuations;
DVE = everything else. Key HW lessons baked in: LDWEIGHTS costs ~40ns per
stationary-operand change (group matmuls sharing lhsT); PSUM bank count
is the binding constraint on attention pipelining; fewer/larger PSUM
evacuations beat many small ones.
"""
import math
import sys
from contextlib import ExitStack

sys.path.insert(0, "/opt/trn_rl_repo")

import numpy as np
import ml_dtypes

import concourse.bass as bass
import concourse.bacc as bacc
import concourse.tile as tile
from concourse import mybir
from concourse.bass_utils import run_bass_kernel_spmd
from concourse.masks import make_identity

F32 = mybir.dt.float32
F32R = mybir.dt.float32r
BF16 = mybir.dt.bfloat16

B, N, DIM, H, D = 64, 260, 768, 12, 64
NCORES = 8
BPC = B // NCORES            # batches per core
KC = DIM // 128              # 6 contraction chunks
QC = [(0, 128), (128, 128), (256, 4)]   # token chunks (offset, size)
NG = 65                      # x-load token-group size (4 groups of 65)
RB = 32                      # bias low-rank factor count (exact rank is <=16)


def _build_program(repeat=1):
    nc = bacc.Bacc("TRN2", target_bir_lowering=False, debug=False,
                   num_devices=NCORES)

    x_d = nc.dram_tensor("x", [BPC, N, DIM], F32, kind="ExternalInput").ap()
    wqkv_d = nc.dram_tensor("wqkv", [DIM, 3 * DIM], F32R, kind="ExternalInput").ap()
    wproj_d = nc.dram_tensor("wproj", [DIM, DIM], F32R, kind="ExternalInput").ap()
    pbrep_d = nc.dram_tensor("pbrep", [128, DIM], F32, kind="ExternalInput").ap()
    # low-rank bias factors: [h, 0] = kv-side (A*sqrt(S))^T, [h, 1] = q-side
    # sqrt(S)*B^T, so bias[kv, q] = sum_r kfac[r, kv] * qfac[r, q]
    bfac_d = nc.dram_tensor("bfac", [H, 2, RB, N], BF16, kind="ExternalInput").ap()
    y_d = nc.dram_tensor("y", [BPC, N, DIM], F32, kind="ExternalOutput").ap()

    with tile.TileContext(nc) as tc, ExitStack() as ctx:
        const = ctx.enter_context(tc.tile_pool(name="const", bufs=1))
        p_x = ctx.enter_context(tc.tile_pool(name="x", bufs=2))
        p_xt = ctx.enter_context(tc.tile_pool(name="xt", bufs=10))
        p_v2 = ctx.enter_context(tc.tile_pool(name="v2", bufs=6))
        p_pr = ctx.enter_context(tc.tile_pool(name="pr", bufs=12))
        # pt2 pool: ONLY chunk-2 prob tiles, in fixed linear-stage order, so
        # its buffer rotation is iteration-periodic (32 allocs % 4 == 0) and
        # the cyclic pipeline's tail linear(0) lands on the same buffers the
        # prologue's linear(0) used
        p_pr2 = ctx.enter_context(tc.tile_pool(name="pr2", bufs=4))
        p_rec = ctx.enter_context(tc.tile_pool(name="rec", bufs=4))
        p_aot = ctx.enter_context(tc.tile_pool(name="aot", bufs=8))
        p_yn = ctx.enter_context(tc.tile_pool(name="yn", bufs=3))
        psum = ctx.enter_context(tc.tile_pool(name="psum", bufs=4, space="PSUM"))
        psum2 = ctx.enter_context(tc.tile_pool(name="psum2", bufs=2, space="PSUM"))

        ident = const.tile([128, 128], F32, tag="ident")
        make_identity(nc, ident)

        wqkv = []
        for kc in range(KC):
            t = const.tile([128, 3 * DIM], F32R, tag=f"wqkv{kc}")
            nc.sync.dma_start(out=t, in_=wqkv_d[128 * kc:128 * (kc + 1), :])
            wqkv.append(t)
        wproj = []
        for kc in range(KC):
            t = const.tile([128, DIM], F32R, tag=f"wproj{kc}")
            nc.sync.dma_start(out=t, in_=wproj_d[128 * kc:128 * (kc + 1), :])
            wproj.append(t)
        # persistent per-head score-operand tiles [64+RB, N] bf16: rows 0:64
        # hold qT/kT for the current batch (rewritten per batch by the qk-proj
        # PSUM evacuation); rows 64:64+RB hold the constant low-rank bias
        # factors, so ONE matmul computes scores+bias (extra K rows are free:
        # matmul cost scales with N only).
        # two sets (batch parity) so batch b+1's qk evacuation never waits on
        # batch b's score matmuls still reading the other set
        qe2 = [[const.tile([64 + RB, N], BF16, tag=f"qe{s}_{h}", name=f"qe{s}_{h}")
                for h in range(H)] for s in range(2)]
        ke2 = [[const.tile([64 + RB, N], BF16, tag=f"ke{s}_{h}", name=f"ke{s}_{h}")
                for h in range(H)] for s in range(2)]
        for s in range(2):
            for h in range(H):
                nc.sync.dma_start(out=ke2[s][h][64:64 + RB, :], in_=bfac_d[h, 0])
                nc.sync.dma_start(out=qe2[s][h][64:64 + RB, :], in_=bfac_d[h, 1])
        pbrep = const.tile([128, DIM], F32, tag="pbrep")
        nc.sync.dma_start(out=pbrep, in_=pbrep_d)
        ones64 = const.tile([128, 64], BF16, tag="ones64")
        nc.vector.memset(ones64, 1.0)

        def _linear(b, v2h, pt2h):
              """x load + transpose + q/k proj + v proj for batch b.

              Yields after each pipelineable piece so the driver can
              interleave its emission with the previous batch's attention
              (keeps every engine's FIFO free of head-of-line blocking)."""
              qe, ke = qe2[b % 2], ke2[b % 2]
              # ---- load x as [65, 4*768] in one DMA ----
              xs = p_x.tile([NG, 4 * DIM], F32, tag="xs")
              nc.sync.dma_start(
                  out=xs.rearrange("p (g d) -> p g d", g=4),
                  in_=x_d[b].rearrange("(g p) d -> p g d", g=4),
              )
              # ---- transpose to xT [DIM, N] ----
              xT = []
              for kc in range(KC):
                  ps = psum.tile([128, N], F32, tag="ps", name="pst")
                  for g in range(4):
                      nc.tensor.transpose(
                          ps[:, NG * g:NG * (g + 1)],
                          xs[:, DIM * g + 128 * kc:DIM * g + 128 * (kc + 1)],
                          ident[:NG, :NG],
                      )
                  t = p_xt.tile([128, N], F32R, tag="xt")
                  nc.vector.tensor_copy(t, ps)
                  xT.append(t)
                  yield

              # ---- q/k projection, transposed layout, evacuated per head ----
              # emit q/k tile pairs together so attention pair hp unlocks early
              for mi in range(2 * H * D // 128):   # 12 tiles of 128 rows
                  m = (mi // 2) + 6 * (mi % 2)     # 0,6,1,7,2,8,...
                  ps = psum.tile([128, N], F32, tag="ps")
                  for kc in range(KC):
                      nc.tensor.matmul(
                          ps,
                          wqkv[kc][:, 128 * m:128 * (m + 1)],
                          xT[kc],
                          start=(kc == 0), stop=(kc == KC - 1),
                      )
                  dst = qe if m < 6 else ke
                  h0 = 2 * (m % 6)
                  nc.scalar.copy(dst[h0][0:64, :], ps[0:64, :])
                  nc.scalar.copy(dst[h0 + 1][0:64, :], ps[64:128, :])
                  yield

              # ---- v natural [N, DIM] interleaved with ones: v2 [N, 2*DIM] bf16 ----
              # chunk 2 (4 rows) is replicated at partitions 0/32/64 so the
              # PV-c2 matmuls can pair it with the partition-packed pt2 probs
              v2 = []
              v2h[b] = v2
              for c, (off, pq) in enumerate(QC):
                  t = p_v2.tile([68 if pq == 4 else pq, 2 * DIM], BF16, tag="v2",
                                name=f"v2c{c}")
                  halves = ((0, 512), (512, 256))
                  pss = [psum.tile([pq, nsz], F32, tag="ps", name=f"v2h{i}")
                         for i, (noff, nsz) in enumerate(halves)]
                  for kc in range(KC):
                      # consecutive matmuls share the stationary operand
                      for (noff, nsz), ps in zip(halves, pss):
                          nc.tensor.matmul(
                              ps,
                              xT[kc][:, off:off + pq],
                              wqkv[kc][:, 2 * DIM + noff:2 * DIM + noff + nsz],
                              start=(kc == 0), stop=(kc == KC - 1),
                          )
                  for (noff, nsz), ps in zip(halves, pss):
                      # scatter head blocks of 64 into interleaved [v_h | ones_h]
                      nh = nsz // 64
                      dst = bass.AP(tensor=t.tensor, offset=t.offset + 2 * noff,
                                    ap=[[t.ap[0][0], pq], [128, nh], [1, 64]])
                      nc.vector.tensor_copy(dst, ps.rearrange("p (h d) -> p h d", d=64))
                  ones_dst = bass.AP(tensor=t.tensor, offset=t.offset + 64,
                                     ap=[[t.ap[0][0], pq], [128, H], [1, 64]])
                  ones_src = bass.AP(tensor=ones64.tensor, offset=ones64.offset,
                                     ap=[[ones64.ap[0][0], pq], [0, H], [1, 64]])
                  nc.vector.tensor_copy(ones_dst, ones_src)
                  if pq == 4:
                      nc.sync.dma_start(out=t[32:36, :], in_=t[0:4, :])
                      nc.sync.dma_start(out=t[64:68, :], in_=t[0:4, :])
                  v2.append(t)
                  yield

              # kv-chunk 2 (4 rows) scores for ALL heads, partition-packed
              # and exp'd by single 260-col Act passes (emitted here in the
              # linear stage so they execute during batch b-1's attention;
              # matmul PSUM writes may only start at partition 0/32/64, so
              # 3 heads per 1-bank group tile, 4 groups)
              off2, pkv2 = QC[2]
              pt2 = []
              pt2h[b] = pt2
              for g in range(4):
                  scg = psum.tile([68, N], F32, tag="ps", name=f"sc2g{g}")
                  for j in range(3):
                      h = 3 * g + j
                      nc.tensor.matmul(scg[32 * j:32 * j + 4, :],
                                       ke[h][:, off2:off2 + pkv2],
                                       qe[h], start=True, stop=True)
                  ptg = p_pr2.tile([68, N], BF16, tag="pr2", name=f"pr2g{g}")
                  nc.scalar.activation(ptg, scg,
                                       mybir.ActivationFunctionType.Exp)
                  pt2.append(ptg)
              yield

        def _attn(b, v2, pt2):
              """attention + output projection for batch b (yields per piece)."""
              qe, ke = qe2[b % 2], ke2[b % 2]
              aoT = [p_aot.tile([128, N], F32R, tag="aot", name=f"aot{i}")
                     for i in range(KC)]
              for hp in range(H // 2):
                  h0, h1 = 2 * hp, 2 * hp + 1
                  # kv-chunks 0/1: one 2-bank slot per chunk holding BOTH
                  # heads' scores (h0 at cols 0:260 in bank 0, h1 at cols
                  # 512:772 in bank 1); one exp per chunk covers both
                  # heads. Each score matmul (K=64+RB) adds the bias via
                  # the constant factor rows of qe/ke.
                  pts = []
                  for c, (off, pkv) in enumerate(QC[:2]):
                      sc = psum2.tile([pkv, 1024], F32, tag="sc")
                      nc.tensor.matmul(sc[:, 0:N], ke[h0][:, off:off + pkv],
                                       qe[h0], start=True, stop=True)
                      nc.tensor.matmul(sc[:, 512:512 + N],
                                       ke[h1][:, off:off + pkv],
                                       qe[h1], start=True, stop=True)
                      pt = p_pr.tile([pkv, 2 * N], BF16, tag="pr")
                      nc.scalar.activation(
                          pt, sc.rearrange("p (b q) -> p b q", b=2)[:, :, 0:N],
                          mybir.ActivationFunctionType.Exp)
                      pts.append(pt)

                  pvs = {h: psum.tile([128, N], F32, tag="ps", name=f"pv{h % 2}")
                         for h in (h0, h1)}
                  # accumulation order c0, c2, c1: c2's operands are ready
                  # early but pair 0 may still be waiting on the v2 chunk-2
                  # replica DMA, so don't lead with it
                  for i, h in enumerate((h0, h1)):
                      nc.tensor.matmul(pvs[h], v2[0][:, 128 * h:128 * (h + 1)],
                                       pts[0][:, i * N:(i + 1) * N],
                                       start=True, stop=False)
                  for i, h in enumerate((h0, h1)):
                      r0 = 32 * (h % 3)
                      nc.tensor.matmul(pvs[h],
                                       v2[2][r0:r0 + 4, 128 * h:128 * (h + 1)],
                                       pt2[h // 3][r0:r0 + 4, :],
                                       start=False, stop=False)
                  for i, h in enumerate((h0, h1)):
                      nc.tensor.matmul(pvs[h], v2[1][:, 128 * h:128 * (h + 1)],
                                       pts[1][:, i * N:(i + 1) * N],
                                       start=False, stop=True)
                  for h in (h0, h1):
                      pv = pvs[h]
                      rec = p_rec.tile([64, N], F32, tag="rec")
                      nc.vector.reciprocal(out=rec, in_=pv[64:128, :])
                      nc.vector.tensor_tensor(
                          aoT[h // 2][64 * (h % 2):64 * (h % 2) + 64, :],
                          pv[0:64, :], rec, op=mybir.AluOpType.mult,
                      )
                  yield

              # ---- output projection, natural layout [tok, DIM] ----
              for c, (off, pq) in enumerate(QC):
                  yn = p_yn.tile([pq, DIM], F32, tag="yn")
                  halves = ((0, 512), (512, 256))
                  pss = [psum.tile([pq, nsz], F32, tag="ps", name=f"pj{i}")
                         for i, (noff, nsz) in enumerate(halves)]
                  for kc in range(KC):
                      for (noff, nsz), ps in zip(halves, pss):
                          nc.tensor.matmul(
                              ps,
                              aoT[kc][:, off:off + pq],
                              wproj[kc][:, noff:noff + nsz],
                              start=(kc == 0), stop=(kc == KC - 1),
                          )
                  for (noff, nsz), ps in zip(halves, pss):
                      nc.vector.tensor_tensor(
                          yn[:, noff:noff + nsz], ps,
                          pbrep[0:pq, noff:noff + nsz], op=mybir.AluOpType.add,
                      )
                  nc.sync.dma_start(out=y_d[b, off:off + pq, :], in_=yn)
                  yield

        _SENT = object()

        v2h, pt2h = {}, {}

        def _body():
            # two-stage software pipeline: batch b's attention emission is
            # interleaved (~2-3 linear pieces per attention piece) with
            # batch b+1's linear stage so no engine queue gets head-of-line
            # blocked behind another batch's work. The pipeline is CYCLIC:
            # batch 7's attention partners with linear(0) of the next
            # repeat-loop iteration (cross-stage pools are allocation-order
            # periodic, so iteration 2+ reads land on the right buffers).
            for b in range(BPC):
                ag = _attn(b, *v2h.pop(b))
                lg = _linear((b + 1) % BPC, v2h, pt2h)
                a_done = l_done = False
                step = 0
                while not (a_done and l_done):
                    if not a_done:
                        a_done = next(ag, _SENT) is _SENT
                    if not l_done:
                        for _ in range(3 if step % 2 == 0 else 2):
                            if next(lg, _SENT) is _SENT:
                                l_done = True
                                break
                    step += 1

        def _stash():
            # pair each batch's v2/pt2 for the driver
            for b in list(v2h):
                v2h[b] = (v2h[b], pt2h.pop(b))

        for _ in _linear(0, v2h, pt2h):
            pass
        _stash()
        if repeat == 1:
            _body()
        else:
            with tc.For_i(0, repeat, 1):
                _body()

    nc.compile()
    return nc


_PROGRAM = None


def _get_program():
    global _PROGRAM
    if _PROGRAM is None:
        _PROGRAM = _build_program()
    return _PROGRAM


def _host_prep(x, qkv_w, proj_w, proj_b, mlp_w1, mlp_b1, mlp_w2, rel_table,
               idx_table, r_cutoff):
    """Host-side: bias table MLP + gather; weight layout prep."""
    x = np.asarray(x, np.float32)
    qkv_w = np.asarray(qkv_w, np.float32)
    proj_w = np.asarray(proj_w, np.float32)
    proj_b = np.asarray(proj_b, np.float32)

    # continuous position bias table: exact GELU MLP
    hdn = np.asarray(rel_table, np.float64) @ np.asarray(mlp_w1, np.float64).T \
        + np.asarray(mlp_b1, np.float64)
    from numpy import vectorize
    erf = vectorize(math.erf)
    hdn = 0.5 * hdn * (1.0 + erf(hdn / math.sqrt(2.0)))
    bt = (hdn @ np.asarray(mlp_w2, np.float64).T).astype(np.float32)  # [T, H]

    idx = np.asarray(idx_table, np.int64)
    rc = int(np.asarray(r_cutoff))
    tok = np.arange(N)
    has_bias = (tok[:, None] >= rc) & (tok[None, :] >= rc)          # [q, kv]
    bias = np.where(has_bias[:, :, None], bt[idx], 0.0)             # [q, kv, H]
    # low-rank factorization per head of bias[kv, q]; the idx_table gather of
    # an MLP-generated table is exactly rank<=16, RB=32 gives slack
    bfac = np.zeros((H, 2, RB, N), np.float64)
    for h in range(H):
        U, S, Vt = np.linalg.svd(bias[:, :, h].T)                   # [kv, q]
        r = min(RB, S.shape[0])
        bfac[h, 0, :r] = (U[:, :r] * np.sqrt(S[:r])).T              # kv side
        bfac[h, 1, :r] = np.sqrt(S[:r])[:, None] * Vt[:r]           # q side
    bfac = bfac.astype(ml_dtypes.bfloat16)

    wqkvT = np.ascontiguousarray(qkv_w.T)                           # [DIM, 3*DIM]
    wqkvT = wqkvT.copy()
    wqkvT[:, :DIM] *= np.float32(0.125)                             # fold 1/sqrt(D)
    wprojT = np.ascontiguousarray(proj_w.T)                         # [DIM, DIM]
    pbrep = np.ascontiguousarray(np.tile(proj_b[None, :], (128, 1)))  # [128, DIM]

    return x, wqkvT, wprojT, pbrep, bfac


def kernel(**inputs):
    x, wqkvT, wprojT, pbrep, bfac = _host_prep(**inputs)
    nc = _get_program()
    in_maps = []
    for c in range(NCORES):
        in_maps.append({
            "x": np.ascontiguousarray(x[c * BPC:(c + 1) * BPC]),
            "wqkv": wqkvT,
            "wproj": wprojT,
            "pbrep": pbrep,
            "bfac": bfac,
        })
    last_err = None
    for attempt in range(5):
        try:
            res = run_bass_kernel_spmd(nc, in_maps, list(range(NCORES)))
            break
        except Exception as e:   # rare transient NRT/axon execution failures
            # (observed: "mesh desynced" UNAVAILABLE, sporadic INTERNAL errors;
            # a plain retry after a short pause recovers)
            last_err = e
            import time as _time
            _time.sleep(2.0 + 2.0 * attempt)
    else:
        raise last_err
    y = np.concatenate([res.results[c]["y"] for c in range(NCORES)], axis=0)
    return y.astype(np.float32)



# revision 46
# speedup vs baseline: 1.3344x; 1.3344x over previous
"""Trainium2 Bass kernel for ViT-style attention with continuous relative
position bias (nn_Attention_18554258718870).

Sharding: data-parallel over batch B=64 across 8 NeuronCores (8 batches per
core); weights / bias table replicated.

Host side computes the tiny bias-table MLP (961x2 -> 961x12) and the
idx_table gather; the [kv, h, q] bias tensor is shipped to the device and
added into the attention scores on-chip via identity matmuls.

Device-side structure (per batch of 8 per core):
- x loaded with one DMA as [65, 4*768]; PE transposes to xT [768, 260].
- q/k projection in transposed layout (12 tiles of [128, 260] f32r).
- v projection in natural layout, interleaved per head as [v_h | ones_h]
  bf16 so the PV matmul computes the softmax denominator for free.
- scores: per head-pair and kv-chunk, ONE 2-bank PSUM slot holds both
  heads' [kv_chunk, 260] score tiles (cols 0:260 / 512:772); the bias is
  accumulated by a bf16 identity matmul (start=True) before the K=64 f32r
  score matmuls (the two heads sit in disjoint PE row-groups 0/64).
  ONE exp per chunk (Act) covers both heads -> probs bf16 [*, 520].
- PV accumulates [d | denom] x probs into one PSUM bank per head; DVE
  stages the denominator to SBUF (reciprocal_approx_fast cannot read PSUM
  on HW - it crashes the NEFF), takes the fast reciprocal and scales.
- output projection computes the natural [tok, 768] layout directly
  (lhsT = aoT chunk, both 512/256 halves through adjacent matmuls sharing
  the stationary operand); the projection bias is added during PSUM
  evacuation against a 128-row replicated bias tile; DMA out per chunk.

PSUM: 2x 2-bank score slots + 4x 1-bank general slots (PV pairs +
projection pipeline). Engine balance: Act = exps + qk/xT evacuations;
DVE = everything else. Key HW lessons baked in: LDWEIGHTS costs ~40ns per
stationary-operand change (group matmuls sharing lhsT); PSUM bank count
is the binding constraint on attention pipelining; fewer/larger PSUM
evacuations beat many small ones.
"""
import math
import sys
from contextlib import ExitStack

sys.path.insert(0, "/opt/trn_rl_repo")

import numpy as np
import ml_dtypes

import concourse.bass as bass
import concourse.bacc as bacc
import concourse.tile as tile
from concourse import mybir
from concourse.bass_utils import run_bass_kernel_spmd
from concourse.masks import make_identity

F32 = mybir.dt.float32
F32R = mybir.dt.float32r
BF16 = mybir.dt.bfloat16

B, N, DIM, H, D = 64, 260, 768, 12, 64
NCORES = 8
BPC = B // NCORES            # batches per core
KC = DIM // 128              # 6 contraction chunks
QC = [(0, 128), (128, 128), (256, 4)]   # token chunks (offset, size)
NG = 65                      # x-load token-group size (4 groups of 65)
RB = 32                      # bias low-rank factor count (exact rank is <=16)


def _build_program(repeat=1):
    nc = bacc.Bacc("TRN2", target_bir_lowering=False, debug=False,
                   num_devices=NCORES)

    x_d = nc.dram_tensor("x", [BPC, N, DIM], F32, kind="ExternalInput").ap()
    wqkv_d = nc.dram_tensor("wqkv", [DIM, 3 * DIM], F32R, kind="ExternalInput").ap()
    wproj_d = nc.dram_tensor("wproj", [DIM, DIM], F32R, kind="ExternalInput").ap()
    pbrep_d = nc.dram_tensor("pbrep", [128, DIM], F32, kind="ExternalInput").ap()
    # low-rank bias factors: [h, 0] = kv-side (A*sqrt(S))^T, [h, 1] = q-side
    # sqrt(S)*B^T, so bias[kv, q] = sum_r kfac[r, kv] * qfac[r, q]
    bfac_d = nc.dram_tensor("bfac", [H, 2, RB, N], BF16, kind="ExternalInput").ap()
    y_d = nc.dram_tensor("y", [BPC, N, DIM], F32, kind="ExternalOutput").ap()

    with tile.TileContext(nc) as tc, ExitStack() as ctx:
        const = ctx.enter_context(tc.tile_pool(name="const", bufs=1))
        p_x = ctx.enter_context(tc.tile_pool(name="x", bufs=2))
        p_xt = ctx.enter_context(tc.tile_pool(name="xt", bufs=10))
        p_v2 = ctx.enter_context(tc.tile_pool(name="v2", bufs=6))
        p_pr = ctx.enter_context(tc.tile_pool(name="pr", bufs=14))
        p_rec = ctx.enter_context(tc.tile_pool(name="rec", bufs=4))
        p_aot = ctx.enter_context(tc.tile_pool(name="aot", bufs=8))
        p_yn = ctx.enter_context(tc.tile_pool(name="yn", bufs=3))
        psum = ctx.enter_context(tc.tile_pool(name="psum", bufs=4, space="PSUM"))
        psum2 = ctx.enter_context(tc.tile_pool(name="psum2", bufs=2, space="PSUM"))

        ident = const.tile([128, 128], F32, tag="ident")
        make_identity(nc, ident)

        wqkv = []
        for kc in range(KC):
            t = const.tile([128, 3 * DIM], F32R, tag=f"wqkv{kc}")
            nc.sync.dma_start(out=t, in_=wqkv_d[128 * kc:128 * (kc + 1), :])
            wqkv.append(t)
        wproj = []
        for kc in range(KC):
            t = const.tile([128, DIM], F32R, tag=f"wproj{kc}")
            nc.sync.dma_start(out=t, in_=wproj_d[128 * kc:128 * (kc + 1), :])
            wproj.append(t)
        # persistent per-head score-operand tiles [64+RB, N] bf16: rows 0:64
        # hold qT/kT for the current batch (rewritten per batch by the qk-proj
        # PSUM evacuation); rows 64:64+RB hold the constant low-rank bias
        # factors, so ONE matmul computes scores+bias (extra K rows are free:
        # matmul cost scales with N only).
        # two sets (batch parity) so batch b+1's qk evacuation never waits on
        # batch b's score matmuls still reading the other set
        qe2 = [[const.tile([64 + RB, N], BF16, tag=f"qe{s}_{h}", name=f"qe{s}_{h}")
                for h in range(H)] for s in range(2)]
        ke2 = [[const.tile([64 + RB, N], BF16, tag=f"ke{s}_{h}", name=f"ke{s}_{h}")
                for h in range(H)] for s in range(2)]
        for s in range(2):
            for h in range(H):
                nc.sync.dma_start(out=ke2[s][h][64:64 + RB, :], in_=bfac_d[h, 0])
                nc.sync.dma_start(out=qe2[s][h][64:64 + RB, :], in_=bfac_d[h, 1])
        pbrep = const.tile([128, DIM], F32, tag="pbrep")
        nc.sync.dma_start(out=pbrep, in_=pbrep_d)
        ones64 = const.tile([128, 64], BF16, tag="ones64")
        nc.vector.memset(ones64, 1.0)

        def _linear(b, v2h):
              """x load + transpose + q/k proj + v proj for batch b.

              Yields after each pipelineable piece so the driver can
              interleave its emission with the previous batch's attention
              (keeps every engine's FIFO free of head-of-line blocking)."""
              qe, ke = qe2[b % 2], ke2[b % 2]
              # ---- load x as [65, 4*768] in one DMA ----
              xs = p_x.tile([NG, 4 * DIM], F32, tag="xs")
              nc.sync.dma_start(
                  out=xs.rearrange("p (g d) -> p g d", g=4),
                  in_=x_d[b].rearrange("(g p) d -> p g d", g=4),
              )
              # ---- transpose to xT [DIM, N] ----
              xT = []
              for kc in range(KC):
                  ps = psum.tile([128, N], F32, tag="ps", name="pst")
                  for g in range(4):
                      nc.tensor.transpose(
                          ps[:, NG * g:NG * (g + 1)],
                          xs[:, DIM * g + 128 * kc:DIM * g + 128 * (kc + 1)],
                          ident[:NG, :NG],
                      )
                  t = p_xt.tile([128, N], F32R, tag="xt")
                  nc.vector.tensor_copy(t, ps)
                  xT.append(t)
                  yield

              # ---- q/k projection, transposed layout, evacuated per head ----
              # emit q/k tile pairs together so attention pair hp unlocks early
              for mi in range(2 * H * D // 128):   # 12 tiles of 128 rows
                  m = (mi // 2) + 6 * (mi % 2)     # 0,6,1,7,2,8,...
                  ps = psum.tile([128, N], F32, tag="ps")
                  for kc in range(KC):
                      nc.tensor.matmul(
                          ps,
                          wqkv[kc][:, 128 * m:128 * (m + 1)],
                          xT[kc],
                          start=(kc == 0), stop=(kc == KC - 1),
                      )
                  dst = qe if m < 6 else ke
                  h0 = 2 * (m % 6)
                  nc.scalar.copy(dst[h0][0:64, :], ps[0:64, :])
                  nc.scalar.copy(dst[h0 + 1][0:64, :], ps[64:128, :])
                  yield

              # ---- v natural [N, DIM] interleaved with ones: v2 [N, 2*DIM] bf16 ----
              # chunk 2 (4 rows) is replicated at partitions 0/32/64 so the
              # PV-c2 matmuls can pair it with the partition-packed pt2 probs
              v2 = []
              v2h[b] = v2
              for c, (off, pq) in enumerate(QC):
                  t = p_v2.tile([68 if pq == 4 else pq, 2 * DIM], BF16, tag="v2",
                                name=f"v2c{c}")
                  halves = ((0, 512), (512, 256))
                  pss = [psum.tile([pq, nsz], F32, tag="ps", name=f"v2h{i}")
                         for i, (noff, nsz) in enumerate(halves)]
                  for kc in range(KC):
                      # consecutive matmuls share the stationary operand
                      for (noff, nsz), ps in zip(halves, pss):
                          nc.tensor.matmul(
                              ps,
                              xT[kc][:, off:off + pq],
                              wqkv[kc][:, 2 * DIM + noff:2 * DIM + noff + nsz],
                              start=(kc == 0), stop=(kc == KC - 1),
                          )
                  for (noff, nsz), ps in zip(halves, pss):
                      # scatter head blocks of 64 into interleaved [v_h | ones_h]
                      nh = nsz // 64
                      dst = bass.AP(tensor=t.tensor, offset=t.offset + 2 * noff,
                                    ap=[[t.ap[0][0], pq], [128, nh], [1, 64]])
                      nc.vector.tensor_copy(dst, ps.rearrange("p (h d) -> p h d", d=64))
                  ones_dst = bass.AP(tensor=t.tensor, offset=t.offset + 64,
                                     ap=[[t.ap[0][0], pq], [128, H], [1, 64]])
                  ones_src = bass.AP(tensor=ones64.tensor, offset=ones64.offset,
                                     ap=[[ones64.ap[0][0], pq], [0, H], [1, 64]])
                  nc.vector.tensor_copy(ones_dst, ones_src)
                  if pq == 4:
                      nc.sync.dma_start(out=t[32:36, :], in_=t[0:4, :])
                      nc.sync.dma_start(out=t[64:68, :], in_=t[0:4, :])
                  v2.append(t)
                  yield

        def _attn(b, v2):
              """attention + output projection for batch b (yields per piece)."""
              qe, ke = qe2[b % 2], ke2[b % 2]
              # kv-chunk 2 (4 rows) scores for ALL heads are partition-packed
              # and exp'd by single 260-col Act passes; PV picks them up as
              # its second accumulation step.
              aoT = [p_aot.tile([128, N], F32R, tag="aot", name=f"aot{i}")
                     for i in range(KC)]
              # (matmul PSUM writes may only start at partition 0/32/64, so
              # 3 heads per 1-bank group tile, 4 groups)
              off2, pkv2 = QC[2]
              pt2 = []
              for g in range(4):
                  scg = psum.tile([68, N], F32, tag="ps", name=f"sc2g{g}")
                  for j in range(3):
                      h = 3 * g + j
                      nc.tensor.matmul(scg[32 * j:32 * j + 4, :],
                                       ke[h][:, off2:off2 + pkv2],
                                       qe[h], start=True, stop=True)
                  ptg = p_pr.tile([68, N], BF16, tag="pr2", name=f"pr2g{g}")
                  nc.scalar.activation(ptg, scg,
                                       mybir.ActivationFunctionType.Exp)
                  pt2.append(ptg)
              yield

              for hp in range(H // 2):
                  h0, h1 = 2 * hp, 2 * hp + 1
                  # kv-chunks 0/1: one 2-bank slot per chunk holding BOTH
                  # heads' scores (h0 at cols 0:260 in bank 0, h1 at cols
                  # 512:772 in bank 1); one exp per chunk covers both
                  # heads. Each score matmul (K=64+RB) adds the bias via
                  # the constant factor rows of qe/ke.
                  pts = []
                  for c, (off, pkv) in enumerate(QC[:2]):
                      sc = psum2.tile([pkv, 1024], F32, tag="sc")
                      nc.tensor.matmul(sc[:, 0:N], ke[h0][:, off:off + pkv],
                                       qe[h0], start=True, stop=True)
                      nc.tensor.matmul(sc[:, 512:512 + N],
                                       ke[h1][:, off:off + pkv],
                                       qe[h1], start=True, stop=True)
                      pt = p_pr.tile([pkv, 2 * N], BF16, tag="pr")
                      nc.scalar.activation(
                          pt, sc.rearrange("p (b q) -> p b q", b=2)[:, :, 0:N],
                          mybir.ActivationFunctionType.Exp)
                      pts.append(pt)

                  pvs = {h: psum.tile([128, N], F32, tag="ps", name=f"pv{h % 2}")
                         for h in (h0, h1)}
                  # accumulation order c0, c2, c1: c2's operands are ready
                  # early but pair 0 may still be waiting on the v2 chunk-2
                  # replica DMA, so don't lead with it
                  for i, h in enumerate((h0, h1)):
                      nc.tensor.matmul(pvs[h], v2[0][:, 128 * h:128 * (h + 1)],
                                       pts[0][:, i * N:(i + 1) * N],
                                       start=True, stop=False)
                  for i, h in enumerate((h0, h1)):
                      r0 = 32 * (h % 3)
                      nc.tensor.matmul(pvs[h],
                                       v2[2][r0:r0 + 4, 128 * h:128 * (h + 1)],
                                       pt2[h // 3][r0:r0 + 4, :],
                                       start=False, stop=False)
                  for i, h in enumerate((h0, h1)):
                      nc.tensor.matmul(pvs[h], v2[1][:, 128 * h:128 * (h + 1)],
                                       pts[1][:, i * N:(i + 1) * N],
                                       start=False, stop=True)
                  for h in (h0, h1):
                      pv = pvs[h]
                      rec = p_rec.tile([64, N], F32, tag="rec")
                      ssb = p_rec.tile([64, N], F32, tag="ssb")
                      nc.vector.tensor_copy(ssb, pv[64:128, :])
                      nc.vector.reciprocal_approx_fast(out=rec, in_=ssb)
                      nc.vector.tensor_tensor(
                          aoT[h // 2][64 * (h % 2):64 * (h % 2) + 64, :],
                          pv[0:64, :], rec, op=mybir.AluOpType.mult,
                      )
                  yield

              # ---- output projection, natural layout [tok, DIM] ----
              for c, (off, pq) in enumerate(QC):
                  yn = p_yn.tile([pq, DIM], F32, tag="yn")
                  halves = ((0, 512), (512, 256))
                  pss = [psum.tile([pq, nsz], F32, tag="ps", name=f"pj{i}")
                         for i, (noff, nsz) in enumerate(halves)]
                  for kc in range(KC):
                      for (noff, nsz), ps in zip(halves, pss):
                          nc.tensor.matmul(
                              ps,
                              aoT[kc][:, off:off + pq],
                              wproj[kc][:, noff:noff + nsz],
                              start=(kc == 0), stop=(kc == KC - 1),
                          )
                  for (noff, nsz), ps in zip(halves, pss):
                      nc.vector.tensor_tensor(
                          yn[:, noff:noff + nsz], ps,
                          pbrep[0:pq, noff:noff + nsz], op=mybir.AluOpType.add,
                      )
                  nc.sync.dma_start(out=y_d[b, off:off + pq, :], in_=yn)
                  yield

        _SENT = object()

        def _body():
            # two-stage software pipeline: batch b's attention emission is
            # interleaved (~2 linear pieces per attention piece) with batch
            # b+1's linear stage so no engine queue gets head-of-line
            # blocked behind another batch's work
            v2h = {}
            for _ in _linear(0, v2h):
                pass
            for b in range(BPC):
                ag = _attn(b, v2h.pop(b))
                lg = _linear(b + 1, v2h) if b + 1 < BPC else None
                a_done = False
                l_done = lg is None
                while not (a_done and l_done):
                    if not a_done:
                        a_done = next(ag, _SENT) is _SENT
                    if not l_done:
                        for _ in range(2):
                            if next(lg, _SENT) is _SENT:
                                l_done = True
                                break

        if repeat == 1:
            _body()
        else:
            with tc.For_i(0, repeat, 1):
                _body()

    nc.compile()
    return nc


_PROGRAM = None


def _get_program():
    global _PROGRAM
    if _PROGRAM is None:
        _PROGRAM = _build_program()
    return _PROGRAM


def _host_prep(x, qkv_w, proj_w, proj_b, mlp_w1, mlp_b1, mlp_w2, rel_table,
               idx_table, r_cutoff):
    """Host-side: bias table MLP + gather; weight layout prep."""
    x = np.asarray(x, np.float32)
    qkv_w = np.asarray(qkv_w, np.float32)
    proj_w = np.asarray(proj_w, np.float32)
    proj_b = np.asarray(proj_b, np.float32)

    # continuous position bias table: exact GELU MLP
    hdn = np.asarray(rel_table, np.float64) @ np.asarray(mlp_w1, np.float64).T \
        + np.asarray(mlp_b1, np.float64)
    from numpy import vectorize
    erf = vectorize(math.erf)
    hdn = 0.5 * hdn * (1.0 + erf(hdn / math.sqrt(2.0)))
    bt = (hdn @ np.asarray(mlp_w2, np.float64).T).astype(np.float32)  # [T, H]

    idx = np.asarray(idx_table, np.int64)
    rc = int(np.asarray(r_cutoff))
    tok = np.arange(N)
    has_bias = (tok[:, None] >= rc) & (tok[None, :] >= rc)          # [q, kv]
    bias = np.where(has_bias[:, :, None], bt[idx], 0.0)             # [q, kv, H]
    # low-rank factorization per head of bias[kv, q]; the idx_table gather of
    # an MLP-generated table is exactly rank<=16, RB=32 gives slack
    bfac = np.zeros((H, 2, RB, N), np.float64)
    for h in range(H):
        U, S, Vt = np.linalg.svd(bias[:, :, h].T)                   # [kv, q]
        r = min(RB, S.shape[0])
        bfac[h, 0, :r] = (U[:, :r] * np.sqrt(S[:r])).T              # kv side
        bfac[h, 1, :r] = np.sqrt(S[:r])[:, None] * Vt[:r]           # q side
    bfac = bfac.astype(ml_dtypes.bfloat16)

    wqkvT = np.ascontiguousarray(qkv_w.T)                           # [DIM, 3*DIM]
    wqkvT = wqkvT.copy()
    wqkvT[:, :DIM] *= np.float32(0.125)                             # fold 1/sqrt(D)
    wprojT = np.ascontiguousarray(proj_w.T)                         # [DIM, DIM]
    pbrep = np.ascontiguousarray(np.tile(proj_b[None, :], (128, 1)))  # [128, DIM]

    return x, wqkvT, wprojT, pbrep, bfac


def kernel(**inputs):
    x, wqkvT, wprojT, pbrep, bfac = _host_prep(**inputs)
    nc = _get_program()
    in_maps = []
    for c in range(NCORES):
        in_maps.append({
            "x": np.ascontiguousarray(x[c * BPC:(c + 1) * BPC]),
            "wqkv": wqkvT,
            "wproj": wprojT,
            "pbrep": pbrep,
            "bfac": bfac,
        })
    last_err = None
    for attempt in range(5):
        try:
            res = run_bass_kernel_spmd(nc, in_maps, list(range(NCORES)))
            break
        except Exception as e:   # rare transient NRT/axon execution failures
            # (observed: "mesh desynced" UNAVAILABLE, sporadic INTERNAL errors;
            # a plain retry after a short pause recovers)
            last_err = e
            import time as _time
            _time.sleep(2.0 + 2.0 * attempt)
    else:
        raise last_err
    y = np.concatenate([res.results[c]["y"] for c in range(NCORES)], axis=0)
    return y.astype(np.float32)

